# revision 59
# baseline (speedup 1.0000x reference)
"""Trainium2 Bass kernel for nn_LossFunc_13752485282042 (chamfer + class + KL + histogram loss).

Contract: kernel(**inputs) takes FULL unsharded numpy inputs (B=256) and returns the
full [256] f32 loss vector. Internally shards batch across 8 NeuronCores (pure data
parallel, 32 samples/core) and runs one SPMD Bass/Tile kernel.

Algorithm per sample (N=M=512 points, K=4 kine dims, D=9 classes):
  f1[n,m] = sum_k x[n,k]*y[m,k] - 0.5*|y[m]|^2   (bf16 triple-split, 27 contraction rows)
  argmax_m f1 == argmin_m ||x_n - y_m||^2 ;  min_m d = 2*(0.5|x_n|^2 - max_m f1)
  4 sequential matmuls per orientation into a double-buffered PSUM tile (overlaps
  with the DVE scans of the previous tile, keeps the PE warm), then a single-pass
  fused custom DVE op (MAX_ARGMAX_ANT) per 512-wide chunk computes BOTH the row max
  and the argmax: the op streams the chunk REVERSED, keeps a running max scan,
  flags positions equal to the running max, and max-accumulates (stream_idx+HUGE)
  over hit positions -> first-occurrence argmax (reference tie semantics); the
  running max itself is streamed through a zero-stride out slot whose final value
  is the row max (corner m*==0 fixed up from a saved column-0 strip).
  Class gather terms via GPSIMD indirect_copy (wrapped per-16-partition index lists,
  relayouted through one DRAM bounce for all samples) + DVE multiply-accumulate;
  histogram/classnum via equality masks + selector matmuls; KL via ACT exp-accumulate.
"""

import numpy as np

import concourse.bass as bass
import concourse.bacc as bacc
import concourse.mybir as mybir
from concourse.tile import TileContext
from concourse.bass_utils import run_bass_kernel_spmd

F32 = mybir.dt.float32
BF16 = mybir.dt.bfloat16
U16 = mybir.dt.uint16
AX = mybir.AxisListType
OP = mybir.AluOpType
ACT = mybir.ActivationFunctionType

B, N, K, D, L = 256, 512, 4, 9, 32
KC = 27                   # matmul contraction rows: 6 bf16-split products x 4 dims + 3 w rows
NCORES = 8
BS = B // NCORES          # 32 samples per core
NCH = N // 128            # 4 partition chunks per sample
NB = BS // 8              # 4 gather batches of 8 samples (one per Q7 group)
SG = 8                    # samples per kine-operand DMA group

TRACE = False             # set by test.py to collect a profile
DEBUG = False
LAST_RESULT = None

# ---------------------------------------------------------------------------
# Custom fused DVE op: single-pass row max + argmax (see module docstring).
# ---------------------------------------------------------------------------
HUGE = 65536.0


def _register_max_argmax():
    import concourse.dve_ops as dve_ops
    from concourse.dve_ops import DveOp
    from concourse.dve_spec import (
        AluOp, Idx, Spec, Src0, eq, lower, maxx, scan, select,
    )
    from concourse.dve_uop import DveOpSpec

    name = "MAX_ARGMAX_ANT"
    for op in dve_ops.OPS:
        if op.name == name:
            return op

    r = scan(AluOp.MAX, Src0)
    body = select(eq(Src0, r), Idx + dve_ops.C0, r)

    def _ref(in0, in1, c0, c1, c2):
        x = np.asarray(in0, np.float32)
        P = x.shape[0]
        x = x.reshape(P, -1)
        rr = np.maximum.accumulate(x, axis=1)
        hit = x == rr
        idx = np.broadcast_to(np.arange(x.shape[1], dtype=np.float32), x.shape)
        bod = np.where(hit, idx + np.float32(c0), rr).astype(np.float32)
        return bod, bod.max(axis=1, keepdims=True)

    spec = Spec(body=body, accum=maxx, reference=_ref)
    row = dve_ops._CUSTOM_DVE_ROW_BASE + len(dve_ops.OPS)
    assert row < 0x20
    dve_ops._SUB_OPCODE_FOR_NAME[name] = row
    shas = {}
    for ver in ("v3", "v4"):
        s = DveOpSpec(name=name, opcode=row, uops=lower(spec, ver=ver), rd1_en=False)
        shas[ver] = s.sha(ver)
    op = DveOp(name, spec, subdim=False, uops_sha=shas)
    dve_ops.OPS.append(op)
    dve_ops.CUSTOM_DVE_SPECS[name] = spec
    return op


MAX_ARGMAX = _register_max_argmax()


def _gather_perm():
    """slot i of the wrapped gather order <-> point index n.

    Wrapped consumption reads idxs[16g+q, col] at slot i = 16*col + q; the
    XBAR-transposed DRAM dump stores point n's index at in-sample u16 offset
    w = n, and the per-group read places offset w at (q = w//32, col = w%32).
    Hence n = 32*(i%16) + i//16.
    """
    i = np.arange(512)
    return 32 * (i % 16) + (i // 16)


PERM = _gather_perm()


def _host_consts():
    selq = np.zeros((128, BS), np.float32)
    for s in range(BS):
        selq[4 * s : 4 * s + NCH, s] = 1.0
    selgs = np.zeros((128, BS), np.float32)
    for s in range(BS):
        g = s % 8
        selgs[16 * g : 16 * g + D, s] = 1.0
    gmask = np.zeros((BS, NB), np.float32)
    for s in range(BS):
        gmask[s, s // 8] = 1.0
    selp = np.zeros((128, BS), np.float32)
    for s in range(BS):
        for c in range(NCH):
            selp[32 * c + s, s] = 1.0
    wrep = np.ones((BS, D), np.float32)
    wrep[:, 0] = 2.0
    wrep[:, D - 1] = 100.0
    return {
        "c_ones": np.ones((128, 1), np.float32),
        "c_selq": selq,
        "c_selgs": selgs,
        "c_gmask": gmask,
        "c_selp": selp,
        "c_seln": -selp,
        "c_wrep": wrep,
    }


CONSTS = _host_consts()


def _build_core_program():
    nc = bacc.Bacc()

    NGRP = BS // SG
    xl = nc.declare_dram_parameter("xl", [NGRP, KC, SG * N], BF16, isOutput=False)
    yr = nc.declare_dram_parameter("yr", [NGRP, KC, SG * N], BF16, isOutput=False)
    yl = nc.declare_dram_parameter("yl", [NGRP, KC, SG * N], BF16, isOutput=False)
    xr = nc.declare_dram_parameter("xr", [NGRP, KC, SG * N], BF16, isOutput=False)
    x2d = nc.declare_dram_parameter("x2d", [128, BS * NCH], F32, isOutput=False)
    y2d = nc.declare_dram_parameter("y2d", [128, BS * NCH], F32, isOutput=False)
    c0a = nc.declare_dram_parameter("c0a", [128, BS * NCH], F32, isOutput=False)
    c0b = nc.declare_dram_parameter("c0b", [128, BS * NCH], F32, isOutput=False)
    hcid = nc.declare_dram_parameter("hcid", [128, 128 * D], F32, isOutput=False)
    hcpd = nc.declare_dram_parameter("hcpd", [128, 128 * D], F32, isOutput=False)
    datg = nc.declare_dram_parameter("datg", [8, D, NB * 2 * N], F32, isOutput=False)
    prtg = nc.declare_dram_parameter("prtg", [8, D, NB * 2 * N], F32, isOutput=False)
    mu = nc.declare_dram_parameter("mu", [BS, L], F32, isOutput=False)
    lv = nc.declare_dram_parameter("lv", [BS, L], F32, isOutput=False)
    c_ones = nc.declare_dram_parameter("c_ones", [128, 1], F32, isOutput=False)
    c_selq = nc.declare_dram_parameter("c_selq", [128, BS], F32, isOutput=False)
    c_selgs = nc.declare_dram_parameter("c_selgs", [128, BS], F32, isOutput=False)
    c_gmask = nc.declare_dram_parameter("c_gmask", [BS, NB], F32, isOutput=False)
    c_selp = nc.declare_dram_parameter("c_selp", [128, BS], F32, isOutput=False)
    c_seln = nc.declare_dram_parameter("c_seln", [128, BS], F32, isOutput=False)
    c_wrep = nc.declare_dram_parameter("c_wrep", [BS, D], F32, isOutput=False)
    out = nc.declare_dram_parameter("out", [BS], F32, isOutput=True)
    if DEBUG:
        nc._dbg_idxw = nc.declare_dram_parameter(
            "d_idxw", [128, 2, BS * NCH], U16, isOutput=True
        )
        nc._dbg_idxf = nc.declare_dram_parameter(
            "d_idxf", [2, 128, BS * NCH], F32, isOutput=True
        )

    with TileContext(nc) as tc:
        _emit(nc, tc, xl, yr, yl, xr, x2d, y2d, c0a, c0b, hcid, hcpd,
              datg, prtg,
              mu, lv, c_ones, c_selq, c_selgs, c_gmask, c_selp, c_seln, c_wrep, out)
    nc.finalize()
    return nc


def _rev_seg(ps, c):
    """Reversed [128, 512] view of chunk c of a [128, 4*512] PSUM tile."""
    seg = ps[:, N * c : N * c + N]
    return bass.AP(
        tensor=seg.tensor, offset=seg.offset + (N - 1), ap=[seg.ap[0], [-1, N]]
    )


def _slot0(t, col):
    """Zero-stride [128, N] write AP aimed at column `col` of tile t."""
    slot = t[:, col : col + 1]
    return bass.AP(tensor=slot.tensor, offset=slot.offset, ap=[slot.ap[0], [0, N]])


def _emit(nc, tc, xl, yr, yl, xr, x2d, y2d, c0a, c0b, hcid, hcpd,
          datg, prtg,
          mu, lv, c_ones, c_selq, c_selgs, c_gmask, c_selp, c_seln, c_wrep, out):
    from contextlib import ExitStack

    ctx = ExitStack()
    with ctx:
        singles = ctx.enter_context(tc.tile_pool(name="singles", bufs=1))
        inp = ctx.enter_context(tc.tile_pool(name="inp", bufs=2))
        coll = ctx.enter_context(tc.tile_pool(name="coll", bufs=2))
        dram = ctx.enter_context(tc.tile_pool(name="dram", bufs=1, space="DRAM"))

        # ---------- group-0 kine operand loads FIRST (critical path) ----------
        kine_tiles = []
        for grp in range(BS // SG):
            XLt = inp.tile([32, SG * N], BF16, tag="XLt", name=f"XLt{grp}")
            YRt = inp.tile([32, SG * N], BF16, tag="YRt", name=f"YRt{grp}")
            YLt = inp.tile([32, SG * N], BF16, tag="YLt", name=f"YLt{grp}")
            XRt = inp.tile([32, SG * N], BF16, tag="XRt", name=f"XRt{grp}")
            kine_tiles.append((XLt, YRt, YLt, XRt))
            if grp == 0:
                # sample-0 slices first so the first matmuls start early
                nc.sync.dma_start(out=XLt[0:KC, 0:N], in_=xl[0, :, 0:N])
                nc.sync.dma_start(out=YRt[0:KC, 0:N], in_=yr[0, :, 0:N])
                nc.sync.dma_start(out=XLt[0:KC, N:], in_=xl[0, :, N:])
                nc.sync.dma_start(out=YRt[0:KC, N:], in_=yr[0, :, N:])
                for t, src in ((YLt, yl), (XRt, xr)):
                    nc.sync.dma_start(out=t[0:KC, :], in_=src[grp])

        # ---------- persistent tiles / constants ----------
        ONES128 = singles.tile([128, 1], F32, tag="ONES128")
        SELQ = singles.tile([128, BS], F32, tag="SELQ")
        SELGS = singles.tile([128, BS], F32, tag="SELGS")
        GMASK = singles.tile([BS, NB], F32, tag="GMASK")
        SELP = singles.tile([128, BS], F32, tag="SELP")
        SELN = singles.tile([128, BS], F32, tag="SELN")
        WREP = singles.tile([BS, D], F32, tag="WREP")
        nc.sync.dma_start(out=ONES128, in_=c_ones[:])
        nc.sync.dma_start(out=SELQ, in_=c_selq[:])
        nc.sync.dma_start(out=SELGS, in_=c_selgs[:])
        nc.sync.dma_start(out=GMASK, in_=c_gmask[:])
        nc.sync.dma_start(out=SELP, in_=c_selp[:])
        nc.sync.dma_start(out=SELN, in_=c_seln[:])
        nc.sync.dma_start(out=WREP, in_=c_wrep[:])

        X2T = singles.tile([128, BS * NCH], F32, tag="X2T")
        Y2T = singles.tile([128, BS * NCH], F32, tag="Y2T")
        nc.sync.dma_start(out=X2T, in_=x2d[:])
        nc.sync.dma_start(out=Y2T, in_=y2d[:])

        # per-orientation: zero-stride rowmax slots, argmax accums, column-0 strip
        RMLAST = [
            singles.tile([128, BS * NCH], F32, tag=f"RMLAST{o}", name=f"RMLAST{o}")
            for o in range(2)
        ]
        ACCT = [
            singles.tile([128, BS * NCH], F32, tag=f"ACCT{o}", name=f"ACCT{o}")
            for o in range(2)
        ]
        COL0T = [
            singles.tile([128, BS * NCH], F32, tag=f"COL0T{o}", name=f"COL0T{o}")
            for o in range(2)
        ]
        nc.scalar.dma_start(out=COL0T[0], in_=c0a[:])
        nc.scalar.dma_start(out=COL0T[1], in_=c0b[:])
        IDXF = [
            singles.tile([128, BS * NCH], F32, tag=f"IDXF{o}", name=f"IDXF{o}")
            for o in range(2)
        ]
        RC = [
            singles.tile([128, BS * NCH], F32, tag=f"RC{o}", name=f"RC{o}")
            for o in range(2)
        ]
        hci = singles.tile([128, 128 * D], F32, tag="hci")
        hcp = singles.tile([128, 128 * D], F32, tag="hcp")
        smu = singles.tile([BS, L], F32, tag="smu")
        slv = singles.tile([BS, L], F32, tag="slv")
        nc.scalar.dma_start(out=smu, in_=mu[:])
        nc.scalar.dma_start(out=slv, in_=lv[:])
        klt = singles.tile([BS, 1], F32, tag="klt")
        TDOT = singles.tile([128, NB], F32, tag="TDOT")
        nc.gpsimd.memset(TDOT, 0.0)

        # gather staging: both orientations interleaved per batch:
        # DATB[:, b, 0, :]=cpT, [:, b, 1, :]=ciT ; PRTB = matching partners
        DATB = singles.tile([128, NB * 2 * N], F32, tag="DATB")
        PRTB = singles.tile([128, NB * 2 * N], F32, tag="PRTB")
        nc.gpsimd.memset(DATB, 0.0)
        nc.gpsimd.memset(PRTB, 0.0)
        for dst, src in ((DATB, datg), (PRTB, prtg)):
            for g in range(8):
                nc.scalar.dma_start(out=dst[16 * g : 16 * g + D, :], in_=src[g])

        # merged two-orientation index tiles; XBAR-transposed DRAM bounce
        IDXCB = singles.tile([128, 2, BS * NCH], U16, tag="IDXCB")
        IDXWB = singles.tile([128, NB * 64], U16, tag="IDXWB")
        TTT = [
            singles.tile([128, 128], U16, tag=f"TTT{o}", name=f"TTT{o}")
            for o in range(2)
        ]
        dscr = dram.tile([2, BS, N], U16, tag="dscr", name="dscr")

        # ---------- main per-sample loop ----------
        cnts = []
        N2 = 2 * N

        def emit_gather_consume(b):
            """Indirect gather + dot for group b (index DMAs issued a group ago)."""
            G = coll.tile([128, N2], F32, tag="G", name=f"G_{b}")
            nc.gpsimd.indirect_copy(
                G.rearrange("p (i e) -> p i e", e=1),
                DATB[:, N2 * b : N2 * b + N2],
                IDXWB[:, 64 * b : 64 * b + 64],
                True,
            )
            GS = coll.tile([128, N2], F32, tag="GS", name=f"GS_{b}")
            nc.vector.scalar_tensor_tensor(
                out=GS, in0=G, scalar=1.0, in1=PRTB[:, N2 * b : N2 * b + N2],
                op0=OP.mult, op1=OP.mult,
                accum_out=TDOT[:, b : b + 1],
            )

        with tc.tile_pool(name="psmm", bufs=8, space="PSUM") as psmm:
            for grp in range(BS // SG):
                XLt, YRt, YLt, XRt = kine_tiles[grp]

                for u in range(SG):
                    s = SG * grp + u
                    for o, (LT, RT) in enumerate(((XLt, YRt), (YLt, XRt))):
                        rhs = RT[0:KC, N * u : N * u + N]
                        # one single-bank PSUM tile per chunk: the fused scan of
                        # chunk c chases matmul c at bank granularity
                        pscs = []
                        for c in range(NCH):
                            PSC = psmm.tile([128, N], F32, tag="PSC",
                                            name=f"PSC{s}_{o}_{c}")
                            pscs.append(PSC)
                            nc.tensor.matmul(
                                PSC,
                                LT[0:KC, N * u + 128 * c : N * u + 128 * c + 128],
                                rhs,
                                start=True, stop=True,
                            )
                        for c in range(NCH):
                            nc.vector._custom_dve(
                                MAX_ARGMAX,
                                out=_slot0(RMLAST[o], NCH * s + c),
                                in0=_rev_seg(pscs[c], 0),
                                s0=HUGE,
                                accum_out=ACCT[o][:, NCH * s + c : NCH * s + c + 1],
                            )

                # ---- per-group index extraction + descriptor bounce DMAs ----
                cs = slice(32 * grp, 32 * grp + 32)
                for o in range(2):
                    # m* = 511 - (acc - HUGE); o=1 block offset +512 in DATB
                    nc.vector.tensor_scalar(
                        out=IDXF[o][:, cs], in0=ACCT[o][:, cs],
                        scalar1=HUGE + (N - 1) + o * N, scalar2=-1.0,
                        op0=OP.subtract, op1=OP.mult,
                    )
                    nc.vector.tensor_copy(IDXCB[:, o, cs], IDXF[o][:, cs])
                # bounce: XBAR transpose -> contiguous DRAM dump (sync queue)
                # -> 64B-run reads (scalar queue)
                for o in range(2):
                    nc.sync.dma_start(out=TTT[o], in_=IDXCB[:, o, :], transpose=True)
                    # TT[4s+j, 16bb+q]; point n of sample s -> dscr[o, s, n]
                    nc.sync.dma_start(
                        out=dscr[o, SG * grp : SG * grp + SG, :]
                        .rearrange("u (j c) -> (u j) c", j=NCH),
                        in_=TTT[o][32 * grp : 32 * grp + 32, :],
                    )
                for g in range(8):
                    for o in range(2):
                        nc.scalar.dma_start(
                            out=IDXWB[16 * g : 16 * g + 16,
                                      64 * grp + 32 * o : 64 * grp + 32 * o + 32],
                            in_=dscr[o, SG * grp + g, :]
                            .rearrange("(q w) -> q w", q=16),
                        )
                # ---- per-group rowmax corner fix + chamfer relu (overlapped) ----
                for o, X2 in enumerate((X2T, Y2T)):
                    M = coll.tile([128, 32], F32, tag=f"CM{o}", name=f"CM{o}_{grp}")
                    nc.vector.tensor_scalar(
                        out=M, in0=ACCT[o][:, cs], scalar1=HUGE + (N - 1),
                        scalar2=None, op0=OP.is_equal,
                    )
                    CD = coll.tile([128, 32], F32, tag=f"CDt{o}", name=f"CD{o}_{grp}")
                    nc.vector.tensor_sub(CD, COL0T[o][:, cs], RMLAST[o][:, cs])
                    CDM = coll.tile([128, 32], F32, tag=f"CDM{o}", name=f"CDM{o}_{grp}")
                    nc.vector.tensor_mul(CDM, CD, M)
                    RMX = coll.tile([128, 32], F32, tag=f"RMXt{o}", name=f"RMX{o}_{grp}")
                    nc.vector.tensor_add(RMX, CDM, RMLAST[o][:, cs])
                    nc.vector.scalar_tensor_tensor(
                        out=RC[o][:, cs], in0=RMX, scalar=-1.0, in1=X2[:, cs],
                        op0=OP.mult, op1=OP.add,
                    )
                    nc.vector.tensor_scalar(
                        out=RC[o][:, cs], in0=RC[o][:, cs], scalar1=0.0,
                        scalar2=None, op0=OP.max,
                    )
                if grp + 1 < BS // SG:
                    for t, src in zip(kine_tiles[grp + 1], (xl, yr, yl, xr)):
                        nc.sync.dma_start(out=t[0:KC, :], in_=src[grp + 1])
                # consume the group-before-last's gather (two-group deferral)
                if grp > 1:
                    emit_gather_consume(grp - 2)
                if grp == 1:
                    # ---- KL (inputs loaded up front; overlaps the loop) ----
                    sexp = singles.tile([BS, L], F32, tag="sexp")
                    sumexp = singles.tile([BS, 1], F32, tag="sumexp")
                    nc.scalar.activation(sexp, slv, ACT.Exp, accum_out=sumexp)
                    smu2 = singles.tile([BS, L], F32, tag="smu2")
                    summu2 = singles.tile([BS, 1], F32, tag="summu2")
                    nc.vector.scalar_tensor_tensor(
                        out=smu2, in0=smu, scalar=1.0, in1=smu,
                        op0=OP.mult, op1=OP.mult, accum_out=summu2,
                    )
                    sumlv = singles.tile([BS, 1], F32, tag="sumlv")
                    nc.vector.reduce_sum(sumlv, slv, axis=AX.X)
                    nc.vector.tensor_sub(klt, sumlv, summu2)
                    nc.vector.tensor_sub(klt, klt, sumexp)
                    nc.vector.tensor_scalar(
                        out=klt, in0=klt, scalar1=float(L), scalar2=None, op0=OP.add
                    )
                    nc.vector.tensor_scalar(
                        out=klt, in0=klt, scalar1=-0.5, scalar2=None, op0=OP.mult
                    )
                if grp == 2:
                    nc.sync.dma_start(out=hci, in_=hcid[:])
                    nc.sync.dma_start(out=hcp, in_=hcpd[:])
            # histogram on DVE fills the idle while the tail gathers land
            for name, h in (("i", hci), ("p", hcp)):
                rmx = singles.tile([128, 128], F32, tag=f"rmx{name}",
                                   name=f"rmx{name}")
                nc.vector.reduce_max(
                    rmx, h.rearrange("p (n d) -> p n d", d=D), axis=AX.X
                )
                oh = singles.tile([128, 128 * D], F32, tag=f"oh{name}",
                                  name=f"oh{name}")
                nc.vector.tensor_tensor(
                    out=oh.rearrange("p (d n) -> p n d", d=D),
                    in0=h.rearrange("p (n d) -> p n d", d=D),
                    in1=rmx.to_broadcast([128, 128, D]),
                    op=OP.is_equal,
                )
                cnt = singles.tile([128, D], F32, tag=f"cnt{name}",
                                   name=f"cnt{name}")
                nc.vector.reduce_sum(
                    cnt, oh.rearrange("p (d n) -> p d n", d=D), axis=AX.X
                )
                cnts.append(cnt)
            emit_gather_consume(BS // SG - 2)
            emit_gather_consume(BS // SG - 1)
            if DEBUG:
                nc.sync.dma_start(out=nc._dbg_idxw[:], in_=IDXWB)
                nc.sync.dma_start(out=nc._dbg_idxf[0], in_=IDXF[0])
                nc.sync.dma_start(out=nc._dbg_idxf[1], in_=IDXF[1])

        # ---------- final assembly (everything in [BS, 1] orientation) ----------
        RC1, RC2 = RC
        with tc.tile_pool(name="psend", bufs=1, space="PSUM") as psend:
            pse12 = psend.tile([128, 2], F32, tag="pse12")
            nc.tensor.matmul(pse12[:, 0:1], RC1, ONES128, start=True, stop=True)
            nc.tensor.matmul(pse12[:, 1:2], RC2, ONES128, start=True, stop=True)
            sc12 = singles.tile([128, 2], F32, tag="sc12")
            nc.scalar.copy(sc12, pse12)
            sc1c = singles.tile([128, 1], F32, tag="sc1c")
            nc.vector.tensor_add(sc1c, sc12[:, 0:1], sc12[:, 1:2])
            chps = psend.tile([BS, 1], F32, tag="chps")
            nc.tensor.matmul(chps, SELQ, sc1c, start=True, stop=True)

            tgp = psend.tile([BS, NB], F32, tag="tgp")
            nc.tensor.matmul(tgp, SELGS, TDOT, start=True, stop=True)
            tgm = singles.tile([BS, NB], F32, tag="tgm")
            tg1 = singles.tile([BS, 1], F32, tag="tg1")
            nc.vector.scalar_tensor_tensor(
                out=tgm, in0=tgp, scalar=1.0, in1=GMASK,
                op0=OP.mult, op1=OP.mult, accum_out=tg1,
            )

            psh = psend.tile([BS, D], F32, tag="psh")
            nc.tensor.matmul(psh, SELP, cnts[0], start=True, stop=False)
            nc.tensor.matmul(psh, SELN, cnts[1], start=False, stop=True)
            habs = singles.tile([BS, D], F32, tag="habs")
            nc.scalar.activation(habs, psh, ACT.Abs)
            hw_ = singles.tile([BS, D], F32, tag="hw_")
            cn1 = singles.tile([BS, 1], F32, tag="cn1")
            nc.vector.scalar_tensor_tensor(
                out=hw_, in0=habs, scalar=1.0, in1=WREP,
                op0=OP.mult, op1=OP.mult, accum_out=cn1,
            )

            tot = singles.tile([BS, 1], F32, tag="tot")
            nc.vector.tensor_scalar(
                out=tot, in0=chps, scalar1=2.0 * 0.99, scalar2=None, op0=OP.mult
            )
            nc.vector.scalar_tensor_tensor(
                out=tot, in0=tg1, scalar=-0.99, in1=tot, op0=OP.mult, op1=OP.add
            )
            nc.vector.scalar_tensor_tensor(
                out=tot, in0=cn1, scalar=0.99 * 0.001, in1=tot, op0=OP.mult, op1=OP.add
            )
            nc.vector.scalar_tensor_tensor(
                out=tot, in0=klt, scalar=0.01, in1=tot, op0=OP.mult, op1=OP.add
            )
            nc.sync.dma_start(out=out.rearrange("(s o) -> s o", o=1), in_=tot)


_NC_CACHE = None


def _get_nc():
    global _NC_CACHE
    if _NC_CACHE is None:
        _NC_CACHE = _build_core_program()
    return _NC_CACHE


def _split3(v):
    import ml_dtypes
    a = v.astype(ml_dtypes.bfloat16)
    r = v - a.astype(np.float32)
    b_ = r.astype(ml_dtypes.bfloat16)
    c = (r - b_.astype(np.float32)).astype(ml_dtypes.bfloat16)
    return a, b_, c


def _build_ops(xT, yT):
    """lhsT-source [B, KC, N] (x-splits + ones) and rhs [B, KC, N] (y-splits + w)."""
    import ml_dtypes
    bf = ml_dtypes.bfloat16
    xa, xb, xc = _split3(xT)              # [B, K, N] each
    ya, yb, yc = _split3(yT)
    w = -0.5 * (yT.astype(np.float64) ** 2).sum(axis=1)   # [B, N]
    wa, wb, wc = _split3(w.astype(np.float32))
    lhs = np.empty((B, KC, N), bf)
    rhs = np.empty((B, KC, N), bf)
    xparts = (xa, xa, xb, xb, xa, xc)
    yparts = (ya, yb, ya, yb, yc, ya)
    for p in range(6):
        lhs[:, 4 * p : 4 * p + K, :] = xparts[p]
        rhs[:, 4 * p : 4 * p + K, :] = yparts[p]
    lhs[:, 24:27, :] = np.ones((), bf)
    for j, ws in enumerate((wa, wb, wc)):
        rhs[:, 24 + j, :] = ws
    return lhs, rhs


def _grp_kine(a):
    """[BS, KC, N] -> [NGRP, KC, SG*N] (device tile layout for one core slice)."""
    ngrp = BS // SG
    return np.ascontiguousarray(
        a.reshape(ngrp, SG, KC, N).transpose(0, 2, 1, 3).reshape(ngrp, KC, SG * N)
    )


def _grp_gather2(aT0, aT1):
    """two [BS, D, N] -> [8, D, NB*2*N]: row g holds samples 8b+g, orientations
    interleaved per batch (o=0 -> aT0, o=1 -> aT1)."""
    m = np.stack([aT0, aT1], axis=1)          # [BS, 2, D, N]
    return np.ascontiguousarray(
        m.reshape(NB, 8, 2, D, N).transpose(1, 3, 0, 2, 4).reshape(8, D, NB * 2 * N)
    )


def build_in_maps(inputs):
    ki = np.ascontiguousarray(np.asarray(inputs["kine_input"], dtype=np.float32))
    kp = np.ascontiguousarray(np.asarray(inputs["kine_pred"], dtype=np.float32))
    cli = np.ascontiguousarray(np.asarray(inputs["class_input"], dtype=np.float32))
    clp = np.ascontiguousarray(np.asarray(inputs["class_pred"], dtype=np.float32))
    mu = np.ascontiguousarray(np.asarray(inputs["mu"], dtype=np.float32))
    lv = np.ascontiguousarray(np.asarray(inputs["log_var"], dtype=np.float32))

    kiT = np.ascontiguousarray(ki.transpose(0, 2, 1))
    kpT = np.ascontiguousarray(kp.transpose(0, 2, 1))
    ciT = np.ascontiguousarray(cli.transpose(0, 2, 1))
    cpT = np.ascontiguousarray(clp.transpose(0, 2, 1))
    ciTp = np.ascontiguousarray(ciT[:, :, PERM])
    cpTp = np.ascontiguousarray(cpT[:, :, PERM])

    # 0.5*|x|^2 in the [point-in-chunk, (sample, chunk)] device layout
    x2 = 0.5 * (ki.astype(np.float64) ** 2).sum(axis=2)   # [B, N]
    y2 = 0.5 * (kp.astype(np.float64) ** 2).sum(axis=2)

    def _chunk_layout(a):
        return np.ascontiguousarray(
            a.astype(np.float32).reshape(B // BS, BS, NCH, 128).transpose(0, 3, 1, 2)
            .reshape(B // BS, 128, BS * NCH)
        )

    x2l = _chunk_layout(x2)
    y2l = _chunk_layout(y2)

    # f1[n, 0] per orientation (corner-fix values when argmax == 0)
    kid = ki.astype(np.float64)
    kpd = kp.astype(np.float64)
    c0a = _chunk_layout(np.einsum("bnk,bk->bn", kid, kpd[:, 0, :]) - y2[:, 0:1])
    c0b = _chunk_layout(np.einsum("bnk,bk->bn", kpd, kid[:, 0, :]) - x2[:, 0:1])

    # hist layout: [32c+s, n*D+d] = cli[s, 128c+n, d]
    def _hist(a, sl):
        return np.ascontiguousarray(
            a[sl].reshape(BS, NCH, 128, D).transpose(1, 0, 2, 3).reshape(128, 128 * D)
        )

    xlh, yrh = _build_ops(kiT, kpT)
    ylh, xrh = _build_ops(kpT, kiT)

    in_maps = []
    for c in range(NCORES):
        sl = slice(BS * c, BS * (c + 1))
        in_maps.append(
            {
                "xl": _grp_kine(xlh[sl]), "yr": _grp_kine(yrh[sl]),
                "yl": _grp_kine(ylh[sl]), "xr": _grp_kine(xrh[sl]),
                "x2d": x2l[c], "y2d": y2l[c],
                "c0a": c0a[c], "c0b": c0b[c],
                "hcid": _hist(cli, sl), "hcpd": _hist(clp, sl),
                "datg": _grp_gather2(cpT[sl], ciT[sl]),
                "prtg": _grp_gather2(ciTp[sl], cpTp[sl]),
                "mu": mu[sl], "lv": lv[sl],
                **CONSTS,
            }
        )
    return in_maps


def kernel(**inputs):
    global LAST_RESULT
    in_maps = build_in_maps(inputs)
    nc = _get_nc()
    res = run_bass_kernel_spmd(nc, in_maps, list(range(NCORES)), trace=TRACE)
    LAST_RESULT = res
    outs = [np.asarray(res.results[c]["out"], dtype=np.float32) for c in range(NCORES)]
    return np.concatenate(outs, axis=0)
